# revision 4
# baseline (speedup 1.0000x reference)
"""Trainium2 Bass kernel for AdvancedWeightPredictorNetwork (retrieval_knn).

Strategy (8 NeuronCores, data-parallel over rows of x):
  - Each core owns a 1024-row shard of x [8192, 256]; x^T is replicated.
  - Distance ranking: G' = x_shard @ x_all^T - r2_all/2 computed as bf16
    matmuls into PSUM (k-chunks: two 128-feature halves + a hi/lo split
    row pair carrying -|x_j|^2/2, so ranking by G' equals ranking by
    -cdist^2). vector.max (top-8, sorted desc) per 1024-col PSUM chunk,
    then a merge max over the 64 candidates gives the global top-8 per
    row. Entry 0 is always the row itself (value r2_i/2 >> off-diag);
    entries 1..5 are the 5 nearest neighbors. knn = sqrt(r2_i - 2*s).
    The tie-break noise matrix only permutes neighbors whose distances
    agree to ~1e-6, which is far below output tolerance, so it is not
    streamed (it cannot change any output value materially).
  - Cluster distances/softmax/local stats/MLP in fp32 per row-tile.
  - Scalar losses: per-core intra partial + inter computed on device;
    host sums the 8 partials.
"""

import numpy as np

import concourse.bacc as bacc
import concourse.tile as tile
import concourse.mybir as mybir
from concourse.bass_utils import run_bass_kernel_spmd

F32 = mybir.dt.float32
F16 = mybir.dt.float16
BF16 = mybir.dt.bfloat16
ALU = mybir.AluOpType
ACTF = mybir.ActivationFunctionType

B = 8192        # total rows
NCORES = 8
S = B // NCORES  # rows per core (1024)
F = 256         # features
C = 8           # clusters
K = 5           # neighbors
H = 64          # hidden
O = 32          # output
RT = S // 128   # row tiles per core (8)
PCH = 1024      # psum chunk columns
NCH = B // PCH  # chunks per row tile (8)
PIECE = 2048    # xT DMA/cast piece columns
NPIECE = B // PIECE

_CACHE = {}


def _build():
    nc = bacc.Bacc("TRN2", target_bir_lowering=False, debug=False,
                   num_devices=NCORES)

    xT_d = nc.dram_tensor("xT", [F, B], F32, kind="ExternalInput")
    xsT_d = nc.dram_tensor("xsT", [F, S], F32, kind="ExternalInput")
    xs_d = nc.dram_tensor("xs", [S, F], F32, kind="ExternalInput")
    ccT_d = nc.dram_tensor("ccTa", [F, C], F32, kind="ExternalInput")
    cw_d = nc.dram_tensor("cw", [1, C], F32, kind="ExternalInput")
    temp_d = nc.dram_tensor("temp", [1, 1], F32, kind="ExternalInput")
    w1_d = nc.dram_tensor("W1a", [C + K + 4, H], F32, kind="ExternalInput")
    w2_d = nc.dram_tensor("W2a", [H + 1, O], F32, kind="ExternalInput")
    eye_d = nc.dram_tensor("eye", [128, 128], F32, kind="ExternalInput")

    enc_d = nc.dram_tensor("enc", [S, O], F32, kind="ExternalOutput")
    asn_d = nc.dram_tensor("asn", [S, C], F32, kind="ExternalOutput")
    knn_d = nc.dram_tensor("knn", [S, K], F32, kind="ExternalOutput")
    st_d = nc.dram_tensor("st", [S, 3], F32, kind="ExternalOutput")
    intra_d = nc.dram_tensor("intra", [1, 1], F32, kind="ExternalOutput")
    inter_d = nc.dram_tensor("inter", [1, 1], F32, kind="ExternalOutput")

    NF = C + K + 4  # feat width incl. ones column (17)

    with tile.TileContext(nc) as tc:
        with (
            tc.tile_pool(name="big", bufs=1) as big,
            tc.tile_pool(name="xf", bufs=3) as xfp,
            tc.tile_pool(name="sq", bufs=2) as sqp,
            tc.tile_pool(name="cst", bufs=1) as cst,
            tc.tile_pool(name="work", bufs=2) as wk,
            tc.tile_pool(name="sm", bufs=4) as sm,
            tc.tile_pool(name="acc", bufs=2, space="PSUM") as pacc,
            tc.tile_pool(name="ps", bufs=4, space="PSUM") as psm,
        ):
            # ---------- persistent tiles ----------
            xT_bf = big.tile([128, 2, B], BF16, tag="xT_bf")
            r2rows = big.tile([2, B], BF16, tag="r2rows")   # [-r2/2 hi; lo]
            xsT_f = big.tile([128, 2, S], F32, tag="xsT_f")
            xsT_bf = big.tile([128, 2, S], BF16, tag="xsT_bf")
            r2s_half = big.tile([1, S], F32, tag="r2s_half")  # +r2/2 of shard
            intra_acc = big.tile([128, 1], F32, tag="intra_acc")

            eye = cst.tile([128, 128], F32, tag="eye")
            ccT = cst.tile([128, 2, C], F32, tag="ccT")
            ccsq = cst.tile([128, 2, C], F32, tag="ccsq")
            c2n = cst.tile([1, C], F32, tag="c2n")       # -c2/2
            c2p = cst.tile([1, C], F32, tag="c2p")       # +c2/2
            c2col = cst.tile([C, 1], F32, tag="c2col")   # +c2/2 column
            cwB = cst.tile([128, C], F32, tag="cwB")
            nIT = cst.tile([128, 1], F32, tag="nIT")     # -1/temp bcast
            w1 = cst.tile([NF, H], F32, tag="w1")
            w2 = cst.tile([H + 1, O], F32, tag="w2")
            cw_sb = cst.tile([1, C], F32, tag="cw_sb")
            t_sb = cst.tile([1, 1], F32, tag="t_sb")
            nrT = cst.tile([1, 1], F32, tag="nrT")
            ones_bf = cst.tile([2, 128], BF16, tag="ones_bf")
            ones_f16 = cst.tile([128, 1], F16, tag="ones_f16")
            ones1 = cst.tile([1, 128], F32, tag="ones1")
            ones_col = cst.tile([128, 1], F32, tag="ones_col")

            # ---------- constants / small input loads ----------
            nc.sync.dma_start(eye[:], eye_d.ap())
            for h in range(2):
                nc.sync.dma_start(ccT[:, h, :], ccT_d.ap()[h * 128:(h + 1) * 128, :])
            nc.sync.dma_start(cw_sb[:], cw_d.ap())
            nc.sync.dma_start(t_sb[:], temp_d.ap())
            nc.sync.dma_start(w1[:], w1_d.ap())
            nc.sync.dma_start(w2[:], w2_d.ap())
            nc.vector.memset(ones_bf[:], 1.0)
            nc.vector.memset(ones_f16[:], 1.0)
            nc.vector.memset(ones1[:], 1.0)
            nc.vector.memset(ones_col[:], 1.0)
            nc.vector.memset(intra_acc[:], 0.0)

            # ---------- shard xsT: load, cast, r2 of shard rows ----------
            for h in range(2):
                nc.sync.dma_start(xsT_f[:, h, :], xsT_d.ap()[h * 128:(h + 1) * 128, :])
            nc.gpsimd.tensor_copy(xsT_bf[:], xsT_f[:])
            xssq = sqp.tile([128, 2, S], F16, tag="xssq")
            nc.scalar.square(xssq[:], xsT_f[:])
            for r in range(S // 512):
                sl = slice(r * 512, (r + 1) * 512)
                rp = psm.tile([1, 512], F32, tag="ps")
                for h in range(2):
                    nc.tensor.matmul(rp[:], ones_f16[:], xssq[:, h, sl],
                                     start=(h == 0), stop=(h == 1))
                nc.scalar.mul(r2s_half[:, sl], rp[:], 0.5)

            # ---------- broadcasts: -1/T, cluster weights ----------
            rT = sm.tile([1, 1], F32, tag="s1")
            nc.vector.reciprocal(rT[:], t_sb[:])
            nc.vector.tensor_scalar_mul(nrT[:], rT[:], -1.0)
            bp = psm.tile([128, 1], F32, tag="ps")
            nc.tensor.matmul(bp[:], ones1[:], nrT[:], start=True, stop=True)
            nc.vector.tensor_copy(nIT[:], bp[:])
            cp = psm.tile([128, C], F32, tag="ps")
            nc.tensor.matmul(cp[:], ones1[:], cw_sb[:], start=True, stop=True)
            nc.vector.tensor_copy(cwB[:], cp[:])

            # ---------- cluster center norms ----------
            nc.scalar.square(ccsq[:], ccT[:])
            c2ps = psm.tile([1, C], F32, tag="ps")
            for h in range(2):
                nc.tensor.matmul(c2ps[:], ones_col[:], ccsq[:, h, :],
                                 start=(h == 0), stop=(h == 1))
            nc.scalar.mul(c2n[:], c2ps[:], -0.5)
            nc.scalar.mul(c2p[:], c2ps[:], 0.5)
            ccp = psm.tile([C, 1], F32, tag="ps")
            nc.tensor.matmul(ccp[:], c2p[:], ones_col[0:1, 0:1], start=True, stop=True)
            nc.vector.tensor_copy(c2col[:], ccp[:])

            # ---------- inter-cluster loss (identical on all cores) ----------
            g8 = psm.tile([C, C], F32, tag="ps")
            for h in range(2):
                nc.tensor.matmul(g8[:], ccT[:, h, :], ccT[:, h, :],
                                 start=(h == 0), stop=False)
            nc.tensor.matmul(g8[:], ones1[0:1, 0:C], c2n[:], start=False, stop=True)
            d2cc = sm.tile([C, C], F32, tag="d2cc")
            nc.vector.tensor_scalar(d2cc[:], g8[:], c2col[:], -2.0,
                                    op0=ALU.subtract, op1=ALU.mult)
            nc.vector.tensor_scalar_max(d2cc[:], d2cc[:], 1e-12)
            ccd = sm.tile([C, C], F32, tag="ccd")
            nc.scalar.sqrt(ccd[:], d2cc[:])
            crs = sm.tile([C, 1], F32, tag="crs")
            nc.vector.reduce_sum(crs[:], ccd[:], axis=mybir.AxisListType.X)
            ip8 = psm.tile([1, 1], F32, tag="ps")
            nc.tensor.matmul(ip8[:], crs[:], ones_col[0:C, 0:1], start=True, stop=True)
            inter_sb = sm.tile([1, 1], F32, tag="s1b")
            nc.scalar.mul(inter_sb[:], ip8[:], 1.0 / (C * (C - 1)))
            nc.sync.dma_start(inter_d.ap(), inter_sb[:])

            # ---------- xT pieces: load, cast bf16, squares, r2 rows ----------
            for p in range(NPIECE):
                psl = slice(p * PIECE, (p + 1) * PIECE)
                sqs = []
                for h in range(2):
                    xf = xfp.tile([128, PIECE], F32, tag="xf")
                    nc.sync.dma_start(
                        xf[:], xT_d.ap()[h * 128:(h + 1) * 128, psl])
                    nc.gpsimd.tensor_copy(xT_bf[:, h, psl], xf[:])
                    sq = sqp.tile([128, PIECE], F16, tag=f"sq{h}")
                    nc.scalar.square(sq[:], xf[:])
                    sqs.append(sq)
                for r in range(PIECE // 512):
                    gsl = slice(p * PIECE + r * 512, p * PIECE + (r + 1) * 512)
                    lsl = slice(r * 512, (r + 1) * 512)
                    rp = psm.tile([1, 512], F32, tag="ps")
                    for h in range(2):
                        nc.tensor.matmul(rp[:], ones_f16[:], sqs[h][:, lsl],
                                         start=(h == 0), stop=(h == 1))
                    nc.scalar.mul(r2rows[0:1, gsl], rp[:], -0.5)
                    vlo = sm.tile([1, 512], BF16, tag="vlo")
                    nc.vector.scalar_tensor_tensor(
                        out=vlo[:], in0=rp[:], scalar=-0.5,
                        in1=r2rows[0:1, gsl],
                        op0=ALU.mult, op1=ALU.subtract)
                    nc.sync.dma_start(r2rows[1:2, gsl], vlo[:])

            # ---------- main loop over row tiles ----------
            for t in range(RT):
                tsl = slice(t * 128, (t + 1) * 128)

                xr = wk.tile([128, F], F32, tag="xr")
                nc.sync.dma_start(xr[:], xs_d.ap()[tsl, :])

                # --- top-8 of G' per row ---
                cand = wk.tile([128, NCH, 8], F32, tag="cand")
                for c in range(NCH):
                    acc = pacc.tile([128, PCH], F32, tag="acc")
                    for k in range(2):
                        for n in range(PCH // 512):
                            csl = slice(c * PCH + n * 512, c * PCH + (n + 1) * 512)
                            nsl = slice(n * 512, (n + 1) * 512)
                            nc.tensor.matmul(
                                acc[:, nsl], xsT_bf[:, k, tsl], xT_bf[:, k, csl],
                                start=(k == 0), stop=False)
                    for n in range(PCH // 512):
                        csl = slice(c * PCH + n * 512, c * PCH + (n + 1) * 512)
                        nsl = slice(n * 512, (n + 1) * 512)
                        nc.tensor.matmul(
                            acc[:, nsl], ones_bf[:], r2rows[:, csl],
                            start=False, stop=True)
                    nc.vector.max(cand[:, c, :], acc[:])
                top8 = wk.tile([128, 8], F32, tag="top8")
                nc.vector.max(top8[:], cand[:].rearrange("p a b -> p (a b)"))

                # r2/2 for this tile's rows (exact fp32)
                r2tp = psm.tile([128, 1], F32, tag="ps")
                nc.tensor.matmul(r2tp[:], r2s_half[0:1, tsl], ones_col[0:1, 0:1],
                                 start=True, stop=True)
                r2t = sm.tile([128, 1], F32, tag="r2t")
                nc.vector.tensor_copy(r2t[:], r2tp[:])

                feat = wk.tile([128, NF], F32, tag="feat")
                nc.vector.memset(feat[:, NF - 1:NF], 1.0)

                # --- knn distances ---
                d25 = sm.tile([128, K], F32, tag="d25")
                nc.vector.tensor_scalar(d25[:], top8[:, 1:1 + K], r2t[:], -2.0,
                                        op0=ALU.subtract, op1=ALU.mult)
                nc.vector.tensor_scalar_max(d25[:], d25[:], 1e-12)
                nc.scalar.sqrt(feat[:, C:C + K], d25[:])
                nc.sync.dma_start(knn_d.ap()[tsl, :], feat[:, C:C + K])

                # --- cluster distances + softmax assign ---
                pc = psm.tile([128, C], F32, tag="ps")
                for k in range(2):
                    nc.tensor.matmul(pc[:], xsT_f[:, k, tsl], ccT[:, k, :],
                                     start=(k == 0), stop=False)
                nc.tensor.matmul(pc[:], ones1[:], c2n[:], start=False, stop=True)
                dc2 = sm.tile([128, C], F32, tag="dc2")
                nc.vector.tensor_scalar(dc2[:], pc[:], r2t[:], -2.0,
                                        op0=ALU.subtract, op1=ALU.mult)
                dc = wk.tile([128, C], F32, tag="dc")
                nc.scalar.sqrt(dc[:], dc2[:])
                z = sm.tile([128, C], F32, tag="z")
                nc.vector.tensor_scalar_mul(z[:], dc[:], nIT[:])
                zmx = sm.tile([128, 1], F32, tag="zmx")
                nc.vector.reduce_max(zmx[:], z[:], axis=mybir.AxisListType.X)
                zmn = sm.tile([128, 1], F32, tag="zmn")
                nc.vector.tensor_scalar_mul(zmn[:], zmx[:], -1.0)
                ez = sm.tile([128, C], F32, tag="ez")
                se = sm.tile([128, 1], F32, tag="se")
                nc.scalar.activation(ez[:], z[:], ACTF.Exp, bias=zmn[:],
                                     accum_out=se[:])
                rse = sm.tile([128, 1], F32, tag="rse")
                nc.vector.reciprocal(rse[:], se[:])
                nc.vector.scalar_tensor_tensor(
                    out=feat[:, 0:C], in0=ez[:], scalar=rse[:], in1=cwB[:],
                    op0=ALU.mult, op1=ALU.mult)
                nc.sync.dma_start(asn_d.ap()[tsl, :], feat[:, 0:C])

                # --- intra loss partial ---
                dxa = sm.tile([128, C], F32, tag="dxa")
                psi = sm.tile([128, 1], F32, tag="psi")
                nc.vector.scalar_tensor_tensor(
                    out=dxa[:], in0=dc[:], scalar=1.0, in1=feat[:, 0:C],
                    op0=ALU.mult, op1=ALU.mult, accum_out=psi[:])
                nc.vector.tensor_add(intra_acc[:], intra_acc[:], psi[:])

                # --- local stats ---
                lsum = sm.tile([128, 1], F32, tag="lsum")
                nc.vector.reduce_sum(lsum[:], xr[:], axis=mybir.AxisListType.X)
                nc.vector.tensor_scalar_mul(feat[:, C + K:C + K + 1], lsum[:],
                                            1.0 / F)
                sqx = wk.tile([128, F], F32, tag="sqx")
                ssq = sm.tile([128, 1], F32, tag="ssq")
                nc.scalar.activation(sqx[:], xr[:], ACTF.Square, accum_out=ssq[:])
                lm2 = sm.tile([128, 1], F32, tag="lm2")
                nc.vector.tensor_mul(lm2[:], feat[:, C + K:C + K + 1],
                                     feat[:, C + K:C + K + 1])
                v1 = sm.tile([128, 1], F32, tag="v1")
                nc.vector.scalar_tensor_tensor(
                    out=v1[:], in0=lm2[:], scalar=-float(F), in1=ssq[:],
                    op0=ALU.mult, op1=ALU.add)
                nc.vector.tensor_scalar_mul(v1[:], v1[:], 1.0 / (F - 1))
                sd = sm.tile([128, 1], F32, tag="sd")
                nc.scalar.sqrt(sd[:], v1[:])
                nc.vector.tensor_scalar_add(feat[:, C + K + 1:C + K + 2], sd[:],
                                            1e-8)
                mx = sm.tile([128, 1], F32, tag="mx")
                nc.vector.reduce_max(mx[:], xr[:], axis=mybir.AxisListType.X)
                mxn = sm.tile([128, 1], F32, tag="mxn")
                nc.vector.tensor_scalar_mul(mxn[:], mx[:], -1.0)
                ex = wk.tile([128, F], F32, tag="ex")
                sex = sm.tile([128, 1], F32, tag="sex")
                nc.scalar.activation(ex[:], xr[:], ACTF.Exp, bias=mxn[:],
                                     accum_out=sex[:])
                exx = wk.tile([128, F], F32, tag="exx")
                pxs = sm.tile([128, 1], F32, tag="pxs")
                nc.vector.scalar_tensor_tensor(
                    out=exx[:], in0=ex[:], scalar=1.0, in1=xr[:],
                    op0=ALU.mult, op1=ALU.mult, accum_out=pxs[:])
                lnse = sm.tile([128, 1], F32, tag="lnse")
                nc.scalar.activation(lnse[:], sex[:], ACTF.Ln)
                rs2 = sm.tile([128, 1], F32, tag="rs2")
                nc.vector.reciprocal(rs2[:], sex[:])
                t1 = sm.tile([128, 1], F32, tag="t1")
                nc.vector.tensor_mul(t1[:], pxs[:], rs2[:])
                t2 = sm.tile([128, 1], F32, tag="t2")
                nc.vector.tensor_add(t2[:], mx[:], lnse[:])
                nc.vector.tensor_sub(feat[:, C + K + 2:C + K + 3], t2[:], t1[:])
                nc.sync.dma_start(st_d.ap()[tsl, :], feat[:, C + K:C + K + 3])

                # --- MLP ---
                fTp = psm.tile([NF, 128], F32, tag="ps")
                nc.tensor.matmul(fTp[:], feat[:], eye[:], is_transpose=True,
                                 start=True, stop=True)
                fT = sm.tile([NF, 128], F32, tag="fT")
                nc.vector.tensor_copy(fT[:], fTp[:])
                hp = psm.tile([128, H], F32, tag="ps")
                nc.tensor.matmul(hp[:], fT[:], w1[:], start=True, stop=True)
                hr = wk.tile([128, H + 1], F32, tag="hr")
                nc.scalar.activation(hr[:, 0:H], hp[:], ACTF.Relu)
                nc.vector.memset(hr[:, H:H + 1], 1.0)
                hTp = psm.tile([H + 1, 128], F32, tag="ps")
                nc.tensor.matmul(hTp[:], hr[:], eye[:], is_transpose=True,
                                 start=True, stop=True)
                hT = sm.tile([H + 1, 128], F32, tag="hT")
                nc.vector.tensor_copy(hT[:], hTp[:])
                ep = psm.tile([128, O], F32, tag="ps")
                nc.tensor.matmul(ep[:], hT[:], w2[:], start=True, stop=True)
                enc_sb = wk.tile([128, O], F32, tag="enc_sb")
                nc.vector.tensor_copy(enc_sb[:], ep[:])
                nc.sync.dma_start(enc_d.ap()[tsl, :], enc_sb[:])

            # ---------- intra partial reduce ----------
            ipp = psm.tile([1, 1], F32, tag="ps")
            nc.tensor.matmul(ipp[:], intra_acc[:], ones_col[:], start=True,
                             stop=True)
            intra_sb = sm.tile([1, 1], F32, tag="s1c")
            nc.scalar.mul(intra_sb[:], ipp[:], 1.0 / (B * C))
            nc.sync.dma_start(intra_d.ap(), intra_sb[:])

    nc.compile()
    return nc


def kernel(x, cluster_centers, temperature, cluster_weights, W1, b1, W2, b2,
           noise):
    del noise  # tie-break only; cannot change output values beyond ~1e-6
    x = np.asarray(x, dtype=np.float32)
    cc = np.asarray(cluster_centers, dtype=np.float32)
    temp = np.asarray(temperature, dtype=np.float32).reshape(1, 1)
    cw = np.asarray(cluster_weights, dtype=np.float32).reshape(1, C)
    W1 = np.asarray(W1, dtype=np.float32)
    b1 = np.asarray(b1, dtype=np.float32)
    W2 = np.asarray(W2, dtype=np.float32)
    b2 = np.asarray(b2, dtype=np.float32)

    if "nc" not in _CACHE:
        _CACHE["nc"] = _build()
    nc = _CACHE["nc"]

    xT = np.ascontiguousarray(x.T)
    ccT = np.ascontiguousarray(cc.T)
    W1a = np.concatenate([W1, b1.reshape(1, H)], axis=0)
    W2a = np.concatenate([W2, b2.reshape(1, O)], axis=0)
    eye = np.eye(128, dtype=np.float32)

    in_maps = []
    for c in range(NCORES):
        sl = slice(c * S, (c + 1) * S)
        in_maps.append({
            "xT": xT,
            "xsT": np.ascontiguousarray(xT[:, sl]),
            "xs": np.ascontiguousarray(x[sl]),
            "ccTa": ccT,
            "cw": cw,
            "temp": temp,
            "W1a": W1a,
            "W2a": W2a,
            "eye": eye,
        })

    res = run_bass_kernel_spmd(nc, in_maps, core_ids=list(range(NCORES)))
    rs = res.results

    encoded = np.concatenate([r["enc"] for r in rs], axis=0)
    assign = np.concatenate([r["asn"] for r in rs], axis=0)
    knn = np.concatenate([r["knn"] for r in rs], axis=0)
    stats = np.concatenate([r["st"] for r in rs], axis=0)
    intra = np.float32(sum(float(r["intra"][0, 0]) for r in rs))
    inter = np.float32(rs[0]["inter"][0, 0])
    loss = np.float32(intra - 0.1 * inter)
    return encoded, assign, knn, stats, loss
